# revision 2
# baseline (speedup 1.0000x reference)
"""BitLinear (packed +/-1 linear layer) Trainium2 kernel.

Math: out[b,o] = sum_k a[b,k]*w[o,k] + bias[o], where a/w are +/-1 values
bit-packed LSB-first into bytes (stored as int32 0..255).

Device strategy (8 NeuronCores, data-parallel over batch):
  - Each core gets B/8 = 1024 batch rows; the full weight matrix is
    replicated.
  - Host sends *transposed packed* uint8 tensors (k on partitions), so the
    device unpacks bits directly into the [K, *] layout the PE needs
    (contraction dim on partitions for both operands; the bit-interleaved
    k-order is consistent between A and W so the dot product is unchanged).
  - Unpack: one DVE tensor_scalar per (kp-tile, bit) moves bit i of every
    byte to bit position 6 and masks:  y = (x << (6-i)) & 0x40  (i=7 uses
    >> 1). Byte value 0x40 bitcast to fp8e4 reads as exactly 2.0, i.e.
    unpacked operands take values {0, 2.0} with no dtype-cast op (walrus
    forbids bitVec ops that cast). Ops run on uint16 views: both byte
    lanes' kept bits source from their own byte for these shifts, and the
    16-bit dtype enables the DVE 4x perf mode.
  - fp8e4 DoubleRow matmuls (256-deep contraction per instruction),
    activation tile stationary, N=512 per PSUM bank; psum = 4*M where
    M is the {0,1} binary dot.
  - Identity: with a = 2*alpha-1, w = 2*omega-1 (alpha,omega in {0,1}):
      out = 4*M - 2*rowsum(alpha) - 2*rowsum(omega) + K + bias
          = psum + r2[b] + c[o]
    where r2[b] = -2*popcount_rows(A), c[o] = bias + K - 2*popcount_rows(W)
    are cheap linear-time host precomputes (fp32-exact integers). The
    epilogue is one scalar_tensor_tensor per psum bank:
      out = (psum + r2_per_partition) + c_tile.

Everything is exact: products in {0,4}, fp32 PSUM accumulation of integers
<= 2^14, so the only rounding differences vs the fp32 reference are in the
final bias add (~1 ulp). Measured ~212-222 us/core steady-state (rep-delta
method), at the measured fp8 DoubleRow PE roofline (~205-209 ns issue rate
per N=512 matmul, 1024 matmuls/core, ~163 TFLOP/s/core effective).
"""

import os
import sys

import numpy as np

for _p in ("/opt/trn_rl_repo", "/root/.axon_site/_ro/trn_rl_repo"):
    if os.path.isdir(_p) and _p not in sys.path:
        sys.path.append(_p)

BATCH = 8192
IN_FEATURES = 4096
OUT_FEATURES = 4096
PACKED_LEN = IN_FEATURES // 8  # 512
N_CORES = 8
P = 128

_NC_CACHE: dict = {}
LAST_RESULTS = None  # stash of the most recent BassKernelResults (for test.py)


def build_program(B, O, K, n_devices=N_CORES, o_half=2048, reps=1,
                  mm_reps=1, up_reps=1, out_bufs=3, stage_bufs=4,
                  psum_bufs=2, a2_bufs=1, w2_bufs=2, epi_engine="split",
                  perf_mode_name="DoubleRow"):
    """Emit the per-core Bass/Tile program. SPMD: same program every core.

    reps>1 repeats the whole compute body (identical writes) so test.py can
    measure pure device time as (T(reps=R) - T(reps=1)) / (R - 1).
    mm_reps / up_reps repeat only the matmul block (restarting PSUM
    accumulation, so the last rep wins) / only the unpack ops (idempotent)
    -- engine-rate microbenchmarks via the same delta method."""
    import concourse.bass as bass  # noqa: F401
    import concourse.mybir as mybir
    import concourse.tile as tile
    from concourse import bacc

    KP = K // 8  # packed k rows
    NT = KP // P  # kp tiles (4)
    NK2 = K // 256  # DoubleRow k-pair tiles (16)
    OH = min(O, o_half)  # o columns processed per outer phase
    NH = O // OH
    NOQ = OH // 512  # psum banks per phase
    NB = B // P  # batch tiles
    assert KP % P == 0 and O % OH == 0 and OH % 512 == 0 and B % P == 0
    assert NK2 * 2 == NT * 8

    u8 = mybir.dt.uint8
    u16 = mybir.dt.uint16
    f32 = mybir.dt.float32
    fp8 = mybir.dt.float8e4
    shl = mybir.AluOpType.logical_shift_left
    shr = mybir.AluOpType.logical_shift_right
    band = mybir.AluOpType.bitwise_and
    add = mybir.AluOpType.add

    nc = bacc.Bacc(
        "TRN2",
        target_bir_lowering=False,
        debug=False,
        num_devices=n_devices,
    )

    at_d = nc.dram_tensor("at", [KP, B], u8, kind="ExternalInput").ap()
    wt_d = nc.dram_tensor("wt", [KP, O], u8, kind="ExternalInput").ap()
    c_d = nc.dram_tensor("c_rep", [P, O], f32, kind="ExternalInput").ap()
    r2_d = nc.dram_tensor("r2t", [P, NB], f32, kind="ExternalInput").ap()
    out_d = nc.dram_tensor("out", [B, O], f32, kind="ExternalOutput").ap()

    def unpack_ops(i):
        # Packed byte -> {0x00, 0x40} per byte lane for bit i: move the bit
        # to position 6 and mask (0x40 bitcast to fp8e4 reads as exactly
        # 2.0). Ops run on uint16 views (2 byte-lanes per element): for
        # shifts <= 6 left / 1 right, each kept bit (6 and 14) sources from
        # its own byte, so lanes stay independent under the 0x4040 mask.
        # bitVec ops keep in/out dtype equal (walrus rule) and the 16-bit
        # dtype enables the DVE 4x perf mode.
        return (shr, 1, band, 0x4040) if i == 7 else (shl, 6 - i, band, 0x4040)

    with tile.TileContext(nc) as tc:
        with (
            tc.tile_pool(name="consts", bufs=1) as cpool,
            tc.tile_pool(name="a2", bufs=a2_bufs) as a2pool,
            tc.tile_pool(name="w2", bufs=w2_bufs) as w2pool,
            tc.tile_pool(name="stage", bufs=stage_bufs) as spool,
            tc.tile_pool(name="outs", bufs=out_bufs) as opool,
            tc.tile_pool(name="psum", bufs=psum_bufs, space="PSUM") as ppool,
        ):
            c_rep = cpool.tile([P, O], f32, name="c_rep_t")
            r2t = cpool.tile([P, NB], f32, name="r2t_t")
            consts_loaded = False

            # repeat body for delta timing (rep>0 re-does identical work)
            for rep in range(reps):
              # ---- unpack activations (whole batch shard, kept resident) ----
              a2 = [
                  a2pool.tile([P, 2, B], u8, name=f"a2_{k2}")
                  for k2 in range(NK2)
              ]

              def emit_unpack(h, w2):
                  nonlocal consts_loaded
                  for t in range(NT):
                      wt_st = spool.tile([P, OH], u8, name="wt_st")
                      nc.sync.dma_start(
                          out=wt_st,
                          in_=wt_d[t * P : (t + 1) * P, h * OH : (h + 1) * OH],
                      )
                      if h == 0:
                          at_st = spool.tile([P, B], u8, name="at_st")
                          nc.sync.dma_start(
                              out=at_st, in_=at_d[t * P : (t + 1) * P, :]
                          )
                      if not consts_loaded:
                          # emitted after the first stage DMAs: the epilogue
                          # constants (2 MiB) must not serialize the DMA
                          # queue ahead of the PE-critical first tiles
                          consts_loaded = True
                          nc.sync.dma_start(out=c_rep, in_=c_d)
                          nc.sync.dma_start(out=r2t, in_=r2_d)
                      for _ur in range(up_reps):
                        for i in range(8):
                          op0, s1, op1, s2 = unpack_ops(i)
                          nc.vector.tensor_scalar(
                              out=w2[4 * t + i // 2][:, i % 2, :].bitcast(u16),
                              in0=wt_st.bitcast(u16),
                              scalar1=s1,
                              scalar2=s2,
                              op0=op0,
                              op1=op1,
                          )
                          if h == 0:
                              nc.vector.tensor_scalar(
                                  out=a2[4 * t + i // 2][:, i % 2, :].bitcast(u16),
                                  in0=at_st.bitcast(u16),
                                  scalar1=s1,
                                  scalar2=s2,
                                  op0=op0,
                                  op1=op1,
                              )

              # the first phase's unpack is interleaved (k2-major) so the PE
              # can start as soon as the first a2/w2 pair lands; later
              # phases' w2 slices are double-buffered (w2_bufs=2) so their
              # unpack streams on the DVE while earlier phases' matmuls run
              w2s = {}
              for h in range(NH):
                  w2s[h] = [
                      w2pool.tile([P, 2, OH], u8, name=f"w2_{k2}")
                      for k2 in range(NK2)
                  ]
                  emit_unpack(h, w2s[h])

              for h in range(NH):
                  w2 = w2s[h]
                  # ---- matmul + epilogue ----
                  for b in range(NB):
                      psums = [
                          ppool.tile([P, 512], f32, name=f"ps_{oq}")
                          for oq in range(NOQ)
                      ]
                      for _mr in range(mm_reps):
                        for k2 in range(NK2):
                          lhsT = a2[k2][:, :, b * P : (b + 1) * P].bitcast(fp8)
                          for oq in range(NOQ):
                              nc.tensor.matmul(
                                  psums[oq],
                                  lhsT,
                                  w2[k2][:, :, oq * 512 : (oq + 1) * 512].bitcast(fp8),
                                  start=(k2 == 0),
                                  stop=(k2 == NK2 - 1),
                                  perf_mode=mybir.MatmulPerfMode.DoubleRow,
                              )
                      for oq in range(NOQ):
                          out_st = opool.tile([P, 512], f32, name="out_st")
                          csl = slice(h * OH + oq * 512, h * OH + (oq + 1) * 512)
                          if epi_engine == "split":
                              # keep the DVE queue free for unpack (strict
                              # FIFO: a psum-waiting epilogue at the head
                              # blocks later-queued unpacks): ACT adds the
                              # per-partition r2 while draining PSUM, GPSIMD
                              # (no PSUM access) adds the c row from SBUF
                              tmp = opool.tile([P, 512], f32, name="tmp_st")
                              nc.scalar.activation(
                                  out=tmp,
                                  in_=psums[oq],
                                  func=mybir.ActivationFunctionType.Identity,
                                  bias=r2t[:, b : b + 1],
                              )
                              nc.gpsimd.tensor_tensor(
                                  out=out_st,
                                  in0=tmp,
                                  in1=c_rep[:, csl],
                                  op=add,
                              )
                          else:
                              getattr(nc, epi_engine).scalar_tensor_tensor(
                                  out=out_st,
                                  in0=psums[oq],
                                  scalar=r2t[:, b : b + 1],
                                  in1=c_rep[:, csl],
                                  op0=add,
                                  op1=add,
                              )
                          # store each bank as soon as its epilogue lands so
                          # the final DMAs overlap the remaining epilogues
                          nc.sync.dma_start(
                              out=out_d[b * P : (b + 1) * P, csl],
                              in_=out_st,
                          )

    nc.compile()
    return nc


_POP = np.unpackbits(np.arange(256, dtype=np.uint8)[:, None], axis=1).sum(1)


def _prep_inputs(input_packed, weight_packed, bias, B, O, K, n_cores):
    """Host-side linear-time preprocessing: cast/transpose/shard + popcount
    rank-1 correction terms."""
    NB = B // n_cores // P
    A8 = input_packed.astype(np.uint8)  # [B, KP]
    W8 = weight_packed.astype(np.uint8)  # [O, KP]
    rA = _POP[A8].sum(1, dtype=np.int64)  # [B]
    rW = _POP[W8].sum(1, dtype=np.int64)  # [O]
    c = (bias.astype(np.float64) + K - 2.0 * rW).astype(np.float32)
    c_rep = np.ascontiguousarray(np.broadcast_to(c, (P, O)))
    r2 = (-2.0 * rA).astype(np.float32)
    at_all = np.ascontiguousarray(A8.T)  # [KP, B]
    wt = np.ascontiguousarray(W8.T)  # [KP, O]
    bsh = B // n_cores
    in_maps = []
    for ci in range(n_cores):
        sl = slice(ci * bsh, (ci + 1) * bsh)
        in_maps.append(
            {
                "at": np.ascontiguousarray(at_all[:, sl]),
                "wt": wt,
                "c_rep": c_rep,
                "r2t": np.ascontiguousarray(r2[sl].reshape(NB, P).T),
            }
        )
    return in_maps


def kernel(input_packed, weight_packed, bias):
    global LAST_RESULTS
    from concourse.bass_utils import run_bass_kernel_spmd

    input_packed = np.asarray(input_packed)
    weight_packed = np.asarray(weight_packed)
    bias = np.asarray(bias)
    B, KP = input_packed.shape
    O = weight_packed.shape[0]
    K = KP * 8
    key = (B, O, K, N_CORES)
    if key not in _NC_CACHE:
        _NC_CACHE[key] = build_program(B // N_CORES, O, K, n_devices=N_CORES)
    nc = _NC_CACHE[key]

    in_maps = _prep_inputs(input_packed, weight_packed, bias, B, O, K, N_CORES)
    res = run_bass_kernel_spmd(nc, in_maps, list(range(N_CORES)))
    LAST_RESULTS = res
    out = np.concatenate([res.results[i]["out"] for i in range(N_CORES)], axis=0)
    return np.asarray(out, dtype=np.float32)



# revision 16
# speedup vs baseline: 1.0207x; 1.0207x over previous
"""BitLinear (packed +/-1 linear layer) Trainium2 kernel.

Math: out[b,o] = sum_k a[b,k]*w[o,k] + bias[o], where a/w are +/-1 values
bit-packed LSB-first into bytes (stored as int32 0..255).

Device strategy (8 NeuronCores, data-parallel over batch):
  - Each core gets B/8 = 1024 batch rows; the full weight matrix is
    replicated.
  - Host sends *transposed packed* uint8 tensors (k on partitions), so the
    device unpacks bits directly into the [K, *] layout the PE needs
    (contraction dim on partitions for both operands; the bit-interleaved
    k-order is consistent between A and W so the dot product is unchanged).
  - Unpack: one DVE tensor_scalar per (kp-tile, bit) moves bit i of every
    byte to bit position 6 and masks:  y = (x << (6-i)) & 0x40  (i=7 uses
    >> 1). Byte value 0x40 bitcast to fp8e4 reads as exactly 2.0, i.e.
    unpacked operands take values {0, 2.0} with no dtype-cast op (walrus
    forbids bitVec ops that cast). Ops run on uint16 views: both byte
    lanes' kept bits source from their own byte for these shifts, and the
    16-bit dtype enables the DVE 4x perf mode.
  - fp8e4 DoubleRow matmuls (256-deep contraction per instruction),
    activation tile stationary, N=512 per PSUM bank; psum = 4*M where
    M is the {0,1} binary dot.
  - Identity: with a = 2*alpha-1, w = 2*omega-1 (alpha,omega in {0,1}):
      out = 4*M - 2*rowsum(alpha) - 2*rowsum(omega) + K + bias
          = psum + r2[b] + c[o]
    where r2[b] = -2*popcount_rows(A), c[o] = bias + K - 2*popcount_rows(W)
    are cheap linear-time host precomputes (fp32-exact integers). The
    epilogue is one scalar_tensor_tensor per psum bank:
      out = (psum + r2_per_partition) + c_tile.

Everything is exact: products in {0,4}, fp32 PSUM accumulation of integers
<= 2^14, so the only rounding differences vs the fp32 reference are in the
final bias add (~1 ulp). Measured ~214-218 us/core steady-state (rep-delta
method), at the fp8 DoubleRow PE *streaming* roofline: the moving operand
advances one N-column (a 2-fp8 pair per partition) per PE cycle, so each
N=512 matmul costs ~512 cycles (~209-214 ns) and 1024 matmuls/core pin the
kernel at ~214-219 us regardless of scheduling. Alternatives benched and
rejected (all numerically exact, all within +-2% run noise, see
build_program knobs): k2-inner bank-constant matmul runs, stationary-reuse
8 across both o-halves (h_oq_inner), contiguous per-matmul operand layouts
(contig=1), o_half in {512, 4096}; DoubleRowSwInterleave produces wrong
results on this hardware.
"""

import os
import sys

import numpy as np

for _p in ("/opt/trn_rl_repo", "/root/.axon_site/_ro/trn_rl_repo"):
    if os.path.isdir(_p) and _p not in sys.path:
        sys.path.append(_p)

BATCH = 8192
IN_FEATURES = 4096
OUT_FEATURES = 4096
PACKED_LEN = IN_FEATURES // 8  # 512
N_CORES = 8
P = 128

_NC_CACHE: dict = {}
LAST_RESULTS = None  # stash of the most recent BassKernelResults (for test.py)


def build_program(B, O, K, n_devices=N_CORES, o_half=2048, reps=1,
                  mm_reps=1, up_reps=1, out_bufs=3, stage_bufs=4,
                  psum_bufs=2, a2_bufs=1, w2_bufs=2, epi_engine="split",
                  perf_mode_name="DoubleRow", loop_order="oq_inner",
                  contig=0):
    """Emit the per-core Bass/Tile program. SPMD: same program every core.

    reps>1 repeats the whole compute body (identical writes) so test.py can
    measure pure device time as (T(reps=R) - T(reps=1)) / (R - 1).
    mm_reps / up_reps repeat only the matmul block (restarting PSUM
    accumulation, so the last rep wins) / only the unpack ops (idempotent)
    -- engine-rate microbenchmarks via the same delta method."""
    import concourse.bass as bass  # noqa: F401
    import concourse.mybir as mybir
    import concourse.tile as tile
    from concourse import bacc

    KP = K // 8  # packed k rows
    NT = KP // P  # kp tiles (4)
    NK2 = K // 256  # DoubleRow k-pair tiles (16)
    OH = min(O, o_half)  # o columns processed per outer phase
    NH = O // OH
    NOQ = OH // 512  # psum banks per phase
    NB = B // P  # batch tiles
    assert KP % P == 0 and O % OH == 0 and OH % 512 == 0 and B % P == 0
    assert NK2 * 2 == NT * 8

    u8 = mybir.dt.uint8
    u16 = mybir.dt.uint16
    f32 = mybir.dt.float32
    fp8 = mybir.dt.float8e4
    perf_mode = getattr(mybir.MatmulPerfMode, perf_mode_name)
    shl = mybir.AluOpType.logical_shift_left
    shr = mybir.AluOpType.logical_shift_right
    band = mybir.AluOpType.bitwise_and
    add = mybir.AluOpType.add

    nc = bacc.Bacc(
        "TRN2",
        target_bir_lowering=False,
        debug=False,
        num_devices=n_devices,
    )

    at_d = nc.dram_tensor("at", [KP, B], u8, kind="ExternalInput").ap()
    wt_d = nc.dram_tensor("wt", [KP, O], u8, kind="ExternalInput").ap()
    c_d = nc.dram_tensor("c_rep", [P, O], f32, kind="ExternalInput").ap()
    r2_d = nc.dram_tensor("r2t", [P, NB], f32, kind="ExternalInput").ap()
    out_d = nc.dram_tensor("out", [B, O], f32, kind="ExternalOutput").ap()

    def unpack_ops(i):
        # Packed byte -> {0x00, 0x40} per byte lane for bit i: move the bit
        # to position 6 and mask (0x40 bitcast to fp8e4 reads as exactly
        # 2.0). Ops run on uint16 views (2 byte-lanes per element): for
        # shifts <= 6 left / 1 right, each kept bit (6 and 14) sources from
        # its own byte, so lanes stay independent under the 0x4040 mask.
        # bitVec ops keep in/out dtype equal (walrus rule) and the 16-bit
        # dtype enables the DVE 4x perf mode.
        return (shr, 1, band, 0x4040) if i == 7 else (shl, 6 - i, band, 0x4040)

    with tile.TileContext(nc) as tc:
        with (
            tc.tile_pool(name="consts", bufs=1) as cpool,
            tc.tile_pool(name="a2", bufs=a2_bufs) as a2pool,
            tc.tile_pool(name="w2", bufs=w2_bufs) as w2pool,
            tc.tile_pool(name="stage", bufs=stage_bufs) as spool,
            tc.tile_pool(name="outs", bufs=out_bufs) as opool,
            tc.tile_pool(name="psum", bufs=psum_bufs, space="PSUM") as ppool,
        ):
            c_rep = cpool.tile([P, O], f32, name="c_rep_t")
            r2t = cpool.tile([P, NB], f32, name="r2t_t")
            consts_loaded = False

            # repeat body for delta timing (rep>0 re-does identical work)
            for rep in range(reps):
              # ---- unpack activations (whole batch shard, kept resident) ----
              # contig=1 lays each matmul operand slice out contiguously:
              # a2 [P, NB, 2, 128] (stationary slice = a2[k2][:, b]),
              # w2 [P, NOQ, 2, 512] (moving slice = w2[k2][:, oq])
              if contig:
                  a2 = [
                      a2pool.tile([P, NB, 2, P], u8, name=f"a2_{k2}")
                      for k2 in range(NK2)
                  ]
              else:
                  a2 = [
                      a2pool.tile([P, 2, B], u8, name=f"a2_{k2}")
                      for k2 in range(NK2)
                  ]

              def emit_unpack(h, w2):
                  nonlocal consts_loaded
                  for t in range(NT):
                      wt_st = spool.tile([P, OH], u8, name="wt_st")
                      nc.sync.dma_start(
                          out=wt_st,
                          in_=wt_d[t * P : (t + 1) * P, h * OH : (h + 1) * OH],
                      )
                      if h == 0:
                          at_st = spool.tile([P, B], u8, name="at_st")
                          nc.sync.dma_start(
                              out=at_st, in_=at_d[t * P : (t + 1) * P, :]
                          )
                      if not consts_loaded:
                          # emitted after the first stage DMAs: the epilogue
                          # constants (2 MiB) must not serialize the DMA
                          # queue ahead of the PE-critical first tiles
                          consts_loaded = True
                          nc.sync.dma_start(out=c_rep, in_=c_d)
                          nc.sync.dma_start(out=r2t, in_=r2_d)
                      for _ur in range(up_reps):
                        for i in range(8):
                          op0, s1, op1, s2 = unpack_ops(i)
                          if contig:
                              w_out = w2[4 * t + i // 2][:, :, i % 2, :].bitcast(u16)
                              w_in = wt_st.bitcast(u16).rearrange(
                                  "p (oq c) -> p oq c", oq=NOQ
                              )
                          else:
                              w_out = w2[4 * t + i // 2][:, i % 2, :].bitcast(u16)
                              w_in = wt_st.bitcast(u16)
                          nc.vector.tensor_scalar(
                              out=w_out,
                              in0=w_in,
                              scalar1=s1,
                              scalar2=s2,
                              op0=op0,
                              op1=op1,
                          )
                          if h == 0:
                              if contig:
                                  a_out = a2[4 * t + i // 2][:, :, i % 2, :].bitcast(u16)
                                  a_in = at_st.bitcast(u16).rearrange(
                                      "p (b c) -> p b c", b=NB
                                  )
                              else:
                                  a_out = a2[4 * t + i // 2][:, i % 2, :].bitcast(u16)
                                  a_in = at_st.bitcast(u16)
                              nc.vector.tensor_scalar(
                                  out=a_out,
                                  in0=a_in,
                                  scalar1=s1,
                                  scalar2=s2,
                                  op0=op0,
                                  op1=op1,
                              )

              # the first phase's unpack is interleaved (k2-major) so the PE
              # can start as soon as the first a2/w2 pair lands; later
              # phases' w2 slices are double-buffered (w2_bufs=2) so their
              # unpack streams on the DVE while earlier phases' matmuls run
              w2s = {}
              for h in range(NH):
                  w2s[h] = [
                      w2pool.tile(
                          [P, NOQ, 2, 512] if contig else [P, 2, OH],
                          u8,
                          name=f"w2_{k2}",
                      )
                      for k2 in range(NK2)
                  ]
                  emit_unpack(h, w2s[h])

              def lhsT_of(k2, b):
                  if contig:
                      return a2[k2][:, b].bitcast(fp8)
                  return a2[k2][:, :, b * P : (b + 1) * P].bitcast(fp8)

              def rhs_of(w2, k2, oq):
                  if contig:
                      return w2[k2][:, oq].bitcast(fp8)
                  return w2[k2][:, :, oq * 512 : (oq + 1) * 512].bitcast(fp8)

              def emit_epilogue(h, b, oq, psum):
                          out_st = opool.tile([P, 512], f32, name="out_st")
                          csl = slice(h * OH + oq * 512, h * OH + (oq + 1) * 512)
                          if epi_engine == "split":
                              # keep the DVE queue free for unpack (strict
                              # FIFO: a psum-waiting epilogue at the head
                              # blocks later-queued unpacks): ACT adds the
                              # per-partition r2 while draining PSUM, GPSIMD
                              # (no PSUM access) adds the c row from SBUF
                              tmp = opool.tile([P, 512], f32, name="tmp_st")
                              nc.scalar.activation(
                                  out=tmp,
                                  in_=psum,
                                  func=mybir.ActivationFunctionType.Identity,
                                  bias=r2t[:, b : b + 1],
                              )
                              nc.gpsimd.tensor_tensor(
                                  out=out_st,
                                  in0=tmp,
                                  in1=c_rep[:, csl],
                                  op=add,
                              )
                          else:
                              getattr(nc, epi_engine).scalar_tensor_tensor(
                                  out=out_st,
                                  in0=psum,
                                  scalar=r2t[:, b : b + 1],
                                  in1=c_rep[:, csl],
                                  op0=add,
                                  op1=add,
                              )
                          # store each bank as soon as its epilogue lands so
                          # the final DMAs overlap the remaining epilogues
                          nc.sync.dma_start(
                              out=out_d[b * P : (b + 1) * P, csl],
                              in_=out_st,
                          )

              if loop_order == "h_oq_inner":
                  # reuse-8: one stationary load serves all 8 psum banks
                  # (both o-halves); per-bank epilogue chases each stop
                  for b in range(NB):
                      psums = [
                          ppool.tile([P, 512], f32, name=f"ps_{j}")
                          for j in range(NH * NOQ)
                      ]
                      for _mr in range(mm_reps):
                        for k2 in range(NK2):
                          lhsT = lhsT_of(k2, b)
                          for h in range(NH):
                              for oq in range(NOQ):
                                  nc.tensor.matmul(
                                      psums[h * NOQ + oq],
                                      lhsT,
                                      rhs_of(w2s[h], k2, oq),
                                      start=(k2 == 0),
                                      stop=(k2 == NK2 - 1),
                                      perf_mode=perf_mode,
                                  )
                      for h in range(NH):
                          for oq in range(NOQ):
                              emit_epilogue(h, b, oq, psums[h * NOQ + oq])
              elif loop_order.startswith("group"):
                  # reuse-G: G psum banks rotate per stationary load; 8//G
                  # bank-groups each get a full k2 run. G=1 ~ k2_inner with
                  # global banks; G=8 ~ h_oq_inner.
                  G = int(loop_order[5:])
                  NBANK = NH * NOQ
                  for b in range(NB):
                      psums = [
                          ppool.tile([P, 512], f32, name=f"ps_{j}")
                          for j in range(NBANK)
                      ]
                      for g in range(NBANK // G):
                          for _mr in range(mm_reps):
                            for k2 in range(NK2):
                              lhsT = lhsT_of(k2, b)
                              for j in range(g * G, (g + 1) * G):
                                  h, oq = divmod(j, NOQ)
                                  nc.tensor.matmul(
                                      psums[j],
                                      lhsT,
                                      rhs_of(w2s[h], k2, oq),
                                      start=(k2 == 0),
                                      stop=(k2 == NK2 - 1),
                                      perf_mode=perf_mode,
                                  )
                          for j in range(g * G, (g + 1) * G):
                              h, oq = divmod(j, NOQ)
                              emit_epilogue(h, b, oq, psums[j])
              else:
                for h in range(NH):
                  w2 = w2s[h]
                  # ---- matmul + epilogue ----
                  for b in range(NB):
                      psums = [
                          ppool.tile([P, 512], f32, name=f"ps_{oq}")
                          for oq in range(NOQ)
                      ]
                      if loop_order == "oq_inner":
                          for _mr in range(mm_reps):
                            for k2 in range(NK2):
                              lhsT = lhsT_of(k2, b)
                              for oq in range(NOQ):
                                  nc.tensor.matmul(
                                      psums[oq],
                                      lhsT,
                                      rhs_of(w2, k2, oq),
                                      start=(k2 == 0),
                                      stop=(k2 == NK2 - 1),
                                      perf_mode=perf_mode,
                                  )
                          for oq in range(NOQ):
                              emit_epilogue(h, b, oq, psums[oq])
                      else:  # k2_inner: bank-constant MM runs, epilogue chases
                          for oq in range(NOQ):
                              for _mr in range(mm_reps):
                                for k2 in range(NK2):
                                  nc.tensor.matmul(
                                      psums[oq],
                                      lhsT_of(k2, b),
                                      rhs_of(w2, k2, oq),
                                      start=(k2 == 0),
                                      stop=(k2 == NK2 - 1),
                                      perf_mode=perf_mode,
                                  )
                              emit_epilogue(h, b, oq, psums[oq])

    nc.compile()
    return nc


_POP = np.unpackbits(np.arange(256, dtype=np.uint8)[:, None], axis=1).sum(1)


def _prep_inputs(input_packed, weight_packed, bias, B, O, K, n_cores):
    """Host-side linear-time preprocessing: cast/transpose/shard + popcount
    rank-1 correction terms."""
    NB = B // n_cores // P
    A8 = input_packed.astype(np.uint8)  # [B, KP]
    W8 = weight_packed.astype(np.uint8)  # [O, KP]
    rA = _POP[A8].sum(1, dtype=np.int64)  # [B]
    rW = _POP[W8].sum(1, dtype=np.int64)  # [O]
    c = (bias.astype(np.float64) + K - 2.0 * rW).astype(np.float32)
    c_rep = np.ascontiguousarray(np.broadcast_to(c, (P, O)))
    r2 = (-2.0 * rA).astype(np.float32)
    at_all = np.ascontiguousarray(A8.T)  # [KP, B]
    wt = np.ascontiguousarray(W8.T)  # [KP, O]
    bsh = B // n_cores
    in_maps = []
    for ci in range(n_cores):
        sl = slice(ci * bsh, (ci + 1) * bsh)
        in_maps.append(
            {
                "at": np.ascontiguousarray(at_all[:, sl]),
                "wt": wt,
                "c_rep": c_rep,
                "r2t": np.ascontiguousarray(r2[sl].reshape(NB, P).T),
            }
        )
    return in_maps


def kernel(input_packed, weight_packed, bias):
    global LAST_RESULTS
    from concourse.bass_utils import run_bass_kernel_spmd

    input_packed = np.asarray(input_packed)
    weight_packed = np.asarray(weight_packed)
    bias = np.asarray(bias)
    B, KP = input_packed.shape
    O = weight_packed.shape[0]
    K = KP * 8
    key = (B, O, K, N_CORES)
    if key not in _NC_CACHE:
        _NC_CACHE[key] = build_program(B // N_CORES, O, K, n_devices=N_CORES)
    nc = _NC_CACHE[key]

    in_maps = _prep_inputs(input_packed, weight_packed, bias, B, O, K, N_CORES)
    res = run_bass_kernel_spmd(nc, in_maps, list(range(N_CORES)))
    LAST_RESULTS = res
    out = np.concatenate([res.results[i]["out"] for i in range(N_CORES)], axis=0)
    return np.asarray(out, dtype=np.float32)

